# revision 7
# baseline (speedup 1.0000x reference)
"""DeepseekV3 naive MoE — Trainium2 Bass kernel (8-core expert-parallel).

Strategy:
  * Host (numpy): route (token,k) pairs by expert id (stable sort, capacity
    C=320 like the reference), assign each of the 128 experts to one of
    8 cores x 16 slots (largest-count expert -> largest slot), pack each
    core's tokens into a transposed activation buffer xT [512, R].
  * Device (Bass/Tile, SPMD on 8 cores): per expert slot, grouped GEMM
    gate/up (weights stationary, tokens moving -> psum [f,128 x N]), SiLU
    on ACT, gate*up on DVE (cast bf16), down-proj GEMM accumulating over
    the 1856-dim into psum [h,128 x N], copy out yT [512, R] fp32.
  * Host: un-transpose, gather per (token,k) pair, scale by router weight,
    accumulate over k. Rows that exceed a slot's capacity (stat. ~never)
    are computed on host in fp32.

All GEMMs run in bf16 (fp32 PSUM accumulation). Weights are cast to bf16
on host, which halves the HBM traffic (the binding roofline) and runs the
PE at full rate (fp32 matmul is 4 cycles/row on TRN2).
"""

import os
import numpy as np
import ml_dtypes

BF16 = ml_dtypes.bfloat16

# Problem constants (hardcoded; must match the reference).
E = 128        # experts
I = 1856       # moe intermediate
K = 6          # experts per token
H = 512        # hidden
T = 4096       # tokens
C_REF = 320    # reference per-expert capacity (pairs with pos>=C_REF drop)

NCORES = 8
EPC = 16       # experts per core

# Fixed per-slot capacities (slot j holds the expert with count-rank 8j..8j+7,
# one per core).  Sized from binomial order statistics of uniform-random
# routing (max over 20k trials + margin); overflow falls back to host fp32.
SLOTS = [320, 228, 220, 216, 210, 208, 204, 202,
         200, 198, 196, 192, 190, 188, 186, 182]
OFF = np.concatenate([[0], np.cumsum(SLOTS)[:-1]]).astype(np.int64)
R = int(np.sum(SLOTS))  # 3408 token-rows per core

NBLK = 15                   # 1856 = 14*128 + 64 i-blocks
GU_COLS = 4 * 2 * I         # 14848: 4 h-tiles x 3712 interleaved gate/up cols
WD_COLS = NBLK * 512        # 7680: 15 i-tiles x 512 h-cols

_CACHE = {}

LAST_RESULTS = None  # BassKernelResults of the most recent device run


def _build_program():
    """Build + compile the SPMD Tile program (same program on all 8 cores)."""
    from contextlib import ExitStack
    import concourse.tile as tile
    from concourse import bacc, mybir

    f32 = mybir.dt.float32
    bf16 = mybir.dt.bfloat16

    nc = bacc.Bacc("TRN2", target_bir_lowering=False, debug=False,
                   enable_asserts=False)
    wgu = nc.dram_tensor("wgu", [EPC, 128, GU_COLS], bf16,
                         kind="ExternalInput").ap()
    wd = nc.dram_tensor("wd", [EPC, 128, WD_COLS], bf16,
                        kind="ExternalInput").ap()
    xT = nc.dram_tensor("xT", [4, 128, R], bf16, kind="ExternalInput").ap()
    xT0 = nc.dram_tensor("xT0", [4, 128, SLOTS[0]], bf16,
                         kind="ExternalInput").ap()
    yT = nc.dram_tensor("yT", [4, 128, R], bf16, kind="ExternalOutput").ap()

    with tile.TileContext(nc) as tc, ExitStack() as ctx:
        xpool = ctx.enter_context(tc.tile_pool(name="xp", bufs=1))
        wgupool = ctx.enter_context(tc.tile_pool(name="wgup", bufs=3))
        wdpool = ctx.enter_context(tc.tile_pool(name="wdp", bufs=3))
        ipool = ctx.enter_context(tc.tile_pool(name="ip", bufs=1))
        spool = ctx.enter_context(tc.tile_pool(name="sp", bufs=3))
        ypool = ctx.enter_context(tc.tile_pool(name="yp", bufs=2))
        gups = ctx.enter_context(tc.tile_pool(name="gups", bufs=4,
                                              space="PSUM"))
        dps = ctx.enter_context(tc.tile_pool(name="dps", bufs=1,
                                             space="PSUM"))

        # Slot-0 activations via a small dedicated tensor so the first
        # matmul only waits on ~1MB + the first weight tile.
        xts0 = []
        for h in range(4):
            t = xpool.tile([128, SLOTS[0]], bf16, tag=f"x0_{h}",
                           name=f"xt0_{h}")
            nc.gpsimd.dma_start(out=t, in_=xT0[h])
            xts0.append(t)
        xts_all = []
        first_wgu = wgupool.tile([128, GU_COLS], bf16, tag="wgu",
                                 name="wgu_t0")
        nc.sync.dma_start(out=first_wgu, in_=wgu[0])
        for h in range(4):
            t = xpool.tile([128, R], bf16, tag=f"x{h}", name=f"xt{h}")
            nc.gpsimd.dma_start(out=t, in_=xT[h])
            xts_all.append(t)

        for s in range(EPC):
            Ns = SLOTS[s]
            off = int(OFF[s])

            if s == 0:
                wgu_t = first_wgu
            else:
                wgu_t = wgupool.tile([128, GU_COLS], bf16, tag="wgu")
                nc.sync.dma_start(out=wgu_t, in_=wgu[s])
            wd_t = wdpool.tile([128, WD_COLS], bf16, tag="wd")
            nc.sync.dma_start(out=wd_t, in_=wd[s])
            xts = xts0 if s == 0 else [t[:, off: off + Ns] for t in xts_all]

            # ---- gate/up proj + SiLU*up, i-block by i-block ----
            inter = []
            for m in range(NBLK):
                bp = 128 if m < 14 else 64
                gc = 256 * m          # gate block cols (within one h-tile)
                uc = 256 * m + bp     # up block cols
                pg = gups.tile([128, Ns], f32, tag="ps")
                pu = gups.tile([128, Ns], f32, tag="ps")
                for hh in range(4):
                    base = 3712 * hh
                    nc.tensor.matmul(pg[:bp],
                                     lhsT=wgu_t[:, base + gc: base + gc + bp],
                                     rhs=xts[hh],
                                     start=(hh == 0), stop=(hh == 3))
                for hh in range(4):
                    base = 3712 * hh
                    nc.tensor.matmul(pu[:bp],
                                     lhsT=wgu_t[:, base + uc: base + uc + bp],
                                     rhs=xts[hh],
                                     start=(hh == 0), stop=(hh == 3))
                sil = spool.tile([128, Ns], f32, tag="sil")
                nc.scalar.activation(sil[:bp], pg[:bp],
                                     mybir.ActivationFunctionType.Silu)
                it = ipool.tile([128, Ns], bf16, tag=f"int{m}")
                nc.vector.tensor_mul(it[:bp], sil[:bp], pu[:bp])
                inter.append((it, bp))

            # ---- down proj: accumulate over i-blocks into 4 h-chunk banks ---
            pd = [dps.tile([128, Ns], f32, tag=f"d{c}", name=f"pd{c}") for c in range(4)]
            for m in range(NBLK):
                it, bp = inter[m]
                for c in range(4):
                    col = 512 * m + 128 * c
                    nc.tensor.matmul(pd[c],
                                     lhsT=wd_t[:bp, col: col + 128],
                                     rhs=it[:bp],
                                     start=(m == 0), stop=(m == NBLK - 1))
            for c in range(4):
                yt = ypool.tile([128, Ns], bf16, tag=f"y{c}")
                nc.scalar.copy(yt, pd[c])
                nc.scalar.dma_start(out=yT[c][:, off: off + Ns], in_=yt)

    nc.compile()
    return nc


def _get_program():
    if "nc" not in _CACHE:
        _CACHE["nc"] = _build_program()
    return _CACHE["nc"]


def _pack_weights(w_gate_up, w_down):
    """Reorder + tile + bf16-cast the expert weights for the device layout."""
    # Column permutation interleaving gate/up in 128-col blocks (last 64).
    col_perm = np.empty(2 * I, np.int64)
    p = 0
    for m in range(NBLK):
        bp = 128 if m < 14 else 64
        g0 = 128 * m
        col_perm[p: p + bp] = np.arange(g0, g0 + bp)
        col_perm[p + bp: p + 2 * bp] = np.arange(I + g0, I + g0 + bp)
        p += 2 * bp
    # [E, 512, 3712] -> [E, 128, 4*3712], partition = h % 128, bf16
    gu = np.ascontiguousarray(w_gate_up[:, :, col_perm])
    gu = gu.reshape(E, 4, 128, 2 * I).transpose(0, 2, 1, 3)
    gu = np.ascontiguousarray(gu).reshape(E, 128, GU_COLS).astype(BF16)
    # [E, 1856, 512] -> pad i to 1920 -> [E, 128, 15*512]
    wdp = np.zeros((E, NBLK * 128, 512), np.float32)
    wdp[:, :I] = w_down
    wdp = wdp.reshape(E, NBLK, 128, 512).transpose(0, 2, 1, 3)
    wdp = np.ascontiguousarray(wdp).reshape(E, 128, WD_COLS).astype(BF16)
    return gu, wdp


def kernel(hidden_states, top_k_index, top_k_weights, w_gate_up, w_down):
    global LAST_RESULTS
    from concourse import bass_utils

    hs = np.asarray(hidden_states, np.float32)
    idx = np.asarray(top_k_index).astype(np.int64)
    wts = np.asarray(top_k_weights, np.float32)
    wgu_f = np.asarray(w_gate_up, np.float32)
    wdn_f = np.asarray(w_down, np.float32)

    # ---------------- routing (mirrors the reference exactly) -------------
    N = T * K
    e = idx.reshape(N)
    order = np.argsort(e, kind="stable")
    e_s = e[order]
    tok_s = order // K
    w_s = wts.reshape(N)[order]
    counts = np.bincount(e, minlength=E).astype(np.int64)
    starts = np.concatenate([[0], np.cumsum(counts)[:-1]])
    pos = np.arange(N, dtype=np.int64) - starts[e_s]

    # expert -> (core, slot): rank experts by count desc, deal round-robin
    rank_order = np.argsort(-counts, kind="stable")
    expert_core = np.empty(E, np.int64)
    expert_slot = np.empty(E, np.int64)
    expert_core[rank_order] = np.arange(E) % NCORES
    expert_slot[rank_order] = np.arange(E) // NCORES
    slots_arr = np.asarray(SLOTS, np.int64)
    slot_sz = slots_arr[expert_slot]      # per-expert device capacity
    slot_off = OFF[expert_slot]

    n_dev = np.minimum(counts, slot_sz)   # rows computed on device
    sel = pos < n_dev[e_s]                # pairs handled on device

    # ---------------- pack device inputs ----------------------------------
    xbuf = np.zeros((NCORES, R, H), np.float32)
    xbuf[expert_core[e_s[sel]], slot_off[e_s[sel]] + pos[sel]] = hs[tok_s[sel]]

    gu_all, wd_all = _pack_weights(wgu_f, wdn_f)
    core_experts = rank_order.reshape(EPC, NCORES).T  # [core, slot]

    in_maps = []
    for c in range(NCORES):
        in_maps.append({
            "wgu": np.ascontiguousarray(gu_all[core_experts[c]]),
            "wd": np.ascontiguousarray(wd_all[core_experts[c]]),
            "xT": np.ascontiguousarray(
                xbuf[c].T.astype(BF16).reshape(4, 128, R)),
            "xT0": np.ascontiguousarray(
                xbuf[c, :SLOTS[0]].T.astype(BF16).reshape(4, 128, SLOTS[0])),
        })

    # ---------------- run on the 8 NeuronCores -----------------------------
    nc = _get_program()
    trace = bool(int(os.environ.get("KERNEL_TRACE", "0")))
    res = bass_utils.run_bass_kernel_spmd(
        nc, in_maps, core_ids=list(range(NCORES)), trace=trace)
    LAST_RESULTS = res

    # ---------------- combine on host --------------------------------------
    # y_all: [NCORES*R + 1, H]; last row stays zero for dropped pairs.
    y_all = np.zeros((NCORES * R + 1, H), np.float32)
    for c in range(NCORES):
        y_all[c * R: (c + 1) * R] = res.results[c]["yT"].reshape(H, R).T.astype(np.float32)

    row_of_pair = np.full(N, NCORES * R, np.int64)
    row_of_pair[order[sel]] = (expert_core[e_s[sel]] * R
                               + slot_off[e_s[sel]] + pos[sel])
    rop = row_of_pair.reshape(T, K)

    out = np.zeros((T, H), np.float32)
    for k in range(K):
        out += wts[:, k: k + 1] * y_all[rop[:, k]]

    # ---------------- host fallback for slot overflow ----------------------
    ovf = (~sel) & (pos < C_REF)
    if np.any(ovf):
        oe = e_s[ovf]
        otok = tok_s[ovf]
        ow = w_s[ovf]
        for ex in np.unique(oe):
            m = oe == ex
            X = hs[otok[m]]
            g = X @ wgu_f[ex, :, :I]
            u = X @ wgu_f[ex, :, I:]
            inter = (g / (1.0 + np.exp(-g))) * u
            yv = inter @ wdn_f[ex]
            np.add.at(out, otok[m], ow[m][:, None] * yv)

    return (out, out)


# revision 8
# speedup vs baseline: 1.0758x; 1.0758x over previous
"""DeepseekV3 naive MoE — Trainium2 Bass kernel (8-core expert-parallel).

Strategy:
  * Host (numpy): route (token,k) pairs by expert id (stable sort, capacity
    C=320 like the reference), assign each of the 128 experts to one of
    8 cores x 16 slots (largest-count expert -> largest slot), pack each
    core's tokens into a transposed activation buffer xT [512, R].
  * Device (Bass/Tile, SPMD on 8 cores): per expert slot, grouped GEMM
    gate/up (weights stationary, tokens moving -> psum [f,128 x N]), SiLU
    on ACT, gate*up on DVE (cast bf16), down-proj GEMM accumulating over
    the 1856-dim into psum [h,128 x N], copy out yT [512, R] fp32.
  * Host: un-transpose, gather per (token,k) pair, scale by router weight,
    accumulate over k. Rows that exceed a slot's capacity (stat. ~never)
    are computed on host in fp32.

All GEMMs run in bf16 (fp32 PSUM accumulation). Weights are cast to bf16
on host, which halves the HBM traffic (the binding roofline) and runs the
PE at full rate (fp32 matmul is 4 cycles/row on TRN2).
"""

import os
import numpy as np
import ml_dtypes

BF16 = ml_dtypes.bfloat16

# Problem constants (hardcoded; must match the reference).
E = 128        # experts
I = 1856       # moe intermediate
K = 6          # experts per token
H = 512        # hidden
T = 4096       # tokens
C_REF = 320    # reference per-expert capacity (pairs with pos>=C_REF drop)

NCORES = 8
EPC = 16       # experts per core

# Fixed per-slot capacities (slot j holds the expert with count-rank 8j..8j+7,
# one per core).  Sized from binomial order statistics of uniform-random
# routing (max over 20k trials + margin); overflow falls back to host fp32.
SLOTS = [320, 228, 220, 216, 210, 208, 204, 202,
         200, 198, 196, 192, 190, 188, 186, 182]
OFF = np.concatenate([[0], np.cumsum(SLOTS)[:-1]]).astype(np.int64)
R = int(np.sum(SLOTS))  # 3408 token-rows per core

NBLK = 15                   # 1856 = 14*128 + 64 i-blocks
GU_COLS = 4 * 2 * I         # 14848: 4 h-tiles x 3712 interleaved gate/up cols
WD_COLS = NBLK * 512        # 7680: 15 i-tiles x 512 h-cols

_CACHE = {}

LAST_RESULTS = None  # BassKernelResults of the most recent device run


def _build_program():
    """Build + compile the SPMD Tile program (same program on all 8 cores)."""
    from contextlib import ExitStack
    import concourse.tile as tile
    from concourse import bacc, mybir

    f32 = mybir.dt.float32
    bf16 = mybir.dt.bfloat16

    nc = bacc.Bacc("TRN2", target_bir_lowering=False, debug=False,
                   enable_asserts=False)
    wgu = nc.dram_tensor("wgu", [EPC, 128, GU_COLS], bf16,
                         kind="ExternalInput").ap()
    wd = nc.dram_tensor("wd", [EPC, 128, WD_COLS], bf16,
                        kind="ExternalInput").ap()
    xT = nc.dram_tensor("xT", [4, 128, R], bf16, kind="ExternalInput").ap()
    xT0 = nc.dram_tensor("xT0", [4, 128, SLOTS[0]], bf16,
                         kind="ExternalInput").ap()
    yT = nc.dram_tensor("yT", [4, 128, R], bf16, kind="ExternalOutput").ap()

    with tile.TileContext(nc) as tc, ExitStack() as ctx:
        xpool = ctx.enter_context(tc.tile_pool(name="xp", bufs=1))
        wgupool = ctx.enter_context(tc.tile_pool(name="wgup", bufs=3))
        wdpool = ctx.enter_context(tc.tile_pool(name="wdp", bufs=3))
        ipool = ctx.enter_context(tc.tile_pool(name="ip", bufs=1))
        spool = ctx.enter_context(tc.tile_pool(name="sp", bufs=3))
        ypool = ctx.enter_context(tc.tile_pool(name="yp", bufs=2))
        gups = ctx.enter_context(tc.tile_pool(name="gups", bufs=4,
                                              space="PSUM"))
        dps = ctx.enter_context(tc.tile_pool(name="dps", bufs=1,
                                             space="PSUM"))

        # Slot-0 activations via a small dedicated tensor so the first
        # matmul only waits on ~1MB + the first weight tile.
        xts0 = []
        for h in range(4):
            t = xpool.tile([128, SLOTS[0]], bf16, tag=f"x0_{h}",
                           name=f"xt0_{h}")
            nc.sync.dma_start(out=t, in_=xT0[h])
            xts0.append(t)
        xts_all = []
        first_wgu = wgupool.tile([128, GU_COLS], bf16, tag="wgu",
                                 name="wgu_t0")
        nc.sync.dma_start(out=first_wgu, in_=wgu[0])
        for h in range(4):
            t = xpool.tile([128, R], bf16, tag=f"x{h}", name=f"xt{h}")
            nc.sync.dma_start(out=t, in_=xT[h])
            xts_all.append(t)

        for s in range(EPC):
            Ns = SLOTS[s]
            off = int(OFF[s])

            if s == 0:
                wgu_t = first_wgu
            else:
                wgu_t = wgupool.tile([128, GU_COLS], bf16, tag="wgu")
                nc.sync.dma_start(out=wgu_t, in_=wgu[s])
            wd_t = wdpool.tile([128, WD_COLS], bf16, tag="wd")
            nc.sync.dma_start(out=wd_t, in_=wd[s])
            xts = xts0 if s == 0 else [t[:, off: off + Ns] for t in xts_all]

            # ---- gate/up proj + SiLU*up, i-block by i-block ----
            inter = []
            for m in range(NBLK):
                bp = 128 if m < 14 else 64
                gc = 256 * m          # gate block cols (within one h-tile)
                uc = 256 * m + bp     # up block cols
                pg = gups.tile([128, Ns], f32, tag="ps")
                pu = gups.tile([128, Ns], f32, tag="ps")
                for hh in range(4):
                    base = 3712 * hh
                    nc.tensor.matmul(pg[:bp],
                                     lhsT=wgu_t[:, base + gc: base + gc + bp],
                                     rhs=xts[hh],
                                     start=(hh == 0), stop=(hh == 3))
                for hh in range(4):
                    base = 3712 * hh
                    nc.tensor.matmul(pu[:bp],
                                     lhsT=wgu_t[:, base + uc: base + uc + bp],
                                     rhs=xts[hh],
                                     start=(hh == 0), stop=(hh == 3))
                sil = spool.tile([128, Ns], f32, tag="sil")
                nc.scalar.activation(sil[:bp], pg[:bp],
                                     mybir.ActivationFunctionType.Silu)
                it = ipool.tile([128, Ns], bf16, tag=f"int{m}")
                nc.vector.tensor_mul(it[:bp], sil[:bp], pu[:bp])
                inter.append((it, bp))

            # ---- down proj: accumulate over i-blocks into 4 h-chunk banks ---
            pd = [dps.tile([128, Ns], f32, tag=f"d{c}", name=f"pd{c}") for c in range(4)]
            for m in range(NBLK):
                it, bp = inter[m]
                for c in range(4):
                    col = 512 * m + 128 * c
                    nc.tensor.matmul(pd[c],
                                     lhsT=wd_t[:bp, col: col + 128],
                                     rhs=it[:bp],
                                     start=(m == 0), stop=(m == NBLK - 1))
            for c in range(4):
                yt = ypool.tile([128, Ns], bf16, tag=f"y{c}")
                nc.scalar.copy(yt, pd[c])
                nc.scalar.dma_start(out=yT[c][:, off: off + Ns], in_=yt)

    nc.compile()
    return nc


def _get_program():
    if "nc" not in _CACHE:
        _CACHE["nc"] = _build_program()
    return _CACHE["nc"]


def _pack_weights(w_gate_up, w_down):
    """Reorder + tile + bf16-cast the expert weights for the device layout."""
    # Column permutation interleaving gate/up in 128-col blocks (last 64).
    col_perm = np.empty(2 * I, np.int64)
    p = 0
    for m in range(NBLK):
        bp = 128 if m < 14 else 64
        g0 = 128 * m
        col_perm[p: p + bp] = np.arange(g0, g0 + bp)
        col_perm[p + bp: p + 2 * bp] = np.arange(I + g0, I + g0 + bp)
        p += 2 * bp
    # [E, 512, 3712] -> [E, 128, 4*3712], partition = h % 128, bf16
    gu = np.ascontiguousarray(w_gate_up[:, :, col_perm])
    gu = gu.reshape(E, 4, 128, 2 * I).transpose(0, 2, 1, 3)
    gu = np.ascontiguousarray(gu).reshape(E, 128, GU_COLS).astype(BF16)
    # [E, 1856, 512] -> pad i to 1920 -> [E, 128, 15*512]
    wdp = np.zeros((E, NBLK * 128, 512), np.float32)
    wdp[:, :I] = w_down
    wdp = wdp.reshape(E, NBLK, 128, 512).transpose(0, 2, 1, 3)
    wdp = np.ascontiguousarray(wdp).reshape(E, 128, WD_COLS).astype(BF16)
    return gu, wdp


def kernel(hidden_states, top_k_index, top_k_weights, w_gate_up, w_down):
    global LAST_RESULTS
    from concourse import bass_utils

    hs = np.asarray(hidden_states, np.float32)
    idx = np.asarray(top_k_index).astype(np.int64)
    wts = np.asarray(top_k_weights, np.float32)
    wgu_f = np.asarray(w_gate_up, np.float32)
    wdn_f = np.asarray(w_down, np.float32)

    # ---------------- routing (mirrors the reference exactly) -------------
    N = T * K
    e = idx.reshape(N)
    order = np.argsort(e, kind="stable")
    e_s = e[order]
    tok_s = order // K
    w_s = wts.reshape(N)[order]
    counts = np.bincount(e, minlength=E).astype(np.int64)
    starts = np.concatenate([[0], np.cumsum(counts)[:-1]])
    pos = np.arange(N, dtype=np.int64) - starts[e_s]

    # expert -> (core, slot): rank experts by count desc, deal round-robin
    rank_order = np.argsort(-counts, kind="stable")
    expert_core = np.empty(E, np.int64)
    expert_slot = np.empty(E, np.int64)
    expert_core[rank_order] = np.arange(E) % NCORES
    expert_slot[rank_order] = np.arange(E) // NCORES
    slots_arr = np.asarray(SLOTS, np.int64)
    slot_sz = slots_arr[expert_slot]      # per-expert device capacity
    slot_off = OFF[expert_slot]

    n_dev = np.minimum(counts, slot_sz)   # rows computed on device
    sel = pos < n_dev[e_s]                # pairs handled on device

    # ---------------- pack device inputs ----------------------------------
    xbuf = np.zeros((NCORES, R, H), np.float32)
    xbuf[expert_core[e_s[sel]], slot_off[e_s[sel]] + pos[sel]] = hs[tok_s[sel]]

    gu_all, wd_all = _pack_weights(wgu_f, wdn_f)
    core_experts = rank_order.reshape(EPC, NCORES).T  # [core, slot]

    in_maps = []
    for c in range(NCORES):
        in_maps.append({
            "wgu": np.ascontiguousarray(gu_all[core_experts[c]]),
            "wd": np.ascontiguousarray(wd_all[core_experts[c]]),
            "xT": np.ascontiguousarray(
                xbuf[c].T.astype(BF16).reshape(4, 128, R)),
            "xT0": np.ascontiguousarray(
                xbuf[c, :SLOTS[0]].T.astype(BF16).reshape(4, 128, SLOTS[0])),
        })

    # ---------------- run on the 8 NeuronCores -----------------------------
    nc = _get_program()
    trace = bool(int(os.environ.get("KERNEL_TRACE", "0")))
    res = bass_utils.run_bass_kernel_spmd(
        nc, in_maps, core_ids=list(range(NCORES)), trace=trace)
    LAST_RESULTS = res

    # ---------------- combine on host --------------------------------------
    # y_all: [NCORES*R + 1, H]; last row stays zero for dropped pairs.
    y_all = np.zeros((NCORES * R + 1, H), np.float32)
    for c in range(NCORES):
        y_all[c * R: (c + 1) * R] = res.results[c]["yT"].reshape(H, R).T.astype(np.float32)

    row_of_pair = np.full(N, NCORES * R, np.int64)
    row_of_pair[order[sel]] = (expert_core[e_s[sel]] * R
                               + slot_off[e_s[sel]] + pos[sel])
    rop = row_of_pair.reshape(T, K)

    out = np.zeros((T, H), np.float32)
    for k in range(K):
        out += wts[:, k: k + 1] * y_all[rop[:, k]]

    # ---------------- host fallback for slot overflow ----------------------
    ovf = (~sel) & (pos < C_REF)
    if np.any(ovf):
        oe = e_s[ovf]
        otok = tok_s[ovf]
        ow = w_s[ovf]
        for ex in np.unique(oe):
            m = oe == ex
            X = hs[otok[m]]
            g = X @ wgu_f[ex, :, :I]
            u = X @ wgu_f[ex, :, I:]
            inter = (g / (1.0 + np.exp(-g))) * u
            yv = inter @ wdn_f[ex]
            np.add.at(out, otok[m], ow[m][:, None] * yv)

    return (out, out)
